# revision 6
# baseline (speedup 1.0000x reference)
"""Trainium2 Bass kernel: causal multi-head attention with RoPE + out-proj.

Problem shapes (hardcoded): x [2, 2048, 2048], W_qkv [6144, 2048],
b_qkv [6144], W_out [2048, 2048], b_out [2048]. H=16 heads, D=128.

Sharding over 8 NeuronCores: core c handles batch b = c//4 and head group
g = c%4 (4 heads). Each core computes its heads' attention output and a
partial out-projection over its 512 columns of the contraction; the host
sums the 4 partials per batch (the "all-reduce") and adds biases.

Device-side layout choices (matmuls are [K,M]@[K,N], N=512 free dim):
- host pre-transposes x and the weight shards so contractions land on
  partitions with contiguous DMA reads
- q/k are produced transposed ([dim, token]) in head PAIRS: one 128-row
  tile holds the even dims of both heads, another the odd dims, so RoPE
  is full-width 128-partition DVE math with same-start-partition
  operands (walrus's TensorTensor verifier rejects mixed partition
  offsets); the scores contraction then splits into two K=64 matmuls
  (first-half dims + second-half dims) accumulating in PSUM
- v is produced in natural layout ([token, dim]) so it can be the
  stationary operand of the attention@V matmul
- scores are computed transposed (keys on partitions, queries free);
  softmax skips the max-subtraction (scores here are ~N(0,1), exp can't
  overflow in fp32) so exp needs no per-row bias, the causal mask is a
  multiplicative 0/1 mask after exp, and the denominator comes from a
  ones-vector matmul over the accumulated exp tiles
"""

import math

import numpy as np

import concourse.bass as bass
import concourse.mybir as mybir
from concourse.bass_utils import run_bass_kernel_spmd
from concourse.tile import TileContext
from concourse.vector_clock import ScopedClock

B, T, C = 2, 2048, 2048
H, D = 16, 128
G = 4  # heads per core
NC = 8  # cores
KT = C // 128  # 16 contraction tiles
TT = T // 128  # 16 token tiles
TCH = T // 512  # 4 token chunks

_F32 = mybir.dt.float32


def _install_drain_patch():
    """Walrus in this env rejects >1 sync wait on one CTRL (Drain) inst.

    Tile's tail drain attaches one wait per outstanding logical proc to a
    single Drain; split them across single-wait NOPs on SP instead.
    """

    def _drain_and_barrier(self, tick_clock, wait_clock):
        probe = self.nc.sync.nop(nofuse=True)
        wait_clock.add_sem_waits(
            probe.ins, ScopedClock({None: tick_clock.global_clock})
        )
        si = probe.ins.sync_info
        if si is not None and len(si.on_wait) > 1:
            waits = list(si.on_wait)
            probe.ins.sync_info = mybir.SyncInfo(
                on_wait=waits[:1], on_update=list(si.on_update)
            )
            for w in waits[1:]:
                extra = self.nc.sync.nop(nofuse=True)
                extra.ins.sync_info = mybir.SyncInfo(on_wait=[w], on_update=[])
        self.nc.sync.drain()
        self.nc.all_engine_barrier()
        popped = self.nc._tile_sem_poison_stack.pop()
        assert popped is self._sem_poison
        self.nc.clear_and_free_semaphores(list(self.sems.allocated().values()))
        self.nc.all_engine_barrier()

    TileContext._drain_and_barrier = _drain_and_barrier


_install_drain_patch()


def _split_multiwait(nc):
    """Walrus here allows only one sync wait per instruction: move extras
    onto single-wait NOPs inserted just before, on the same engine."""
    for fn in nc.m.functions:
        for bb in fn.blocks:
            insts = bb.instructions
            new = []
            changed = False
            for inst in insts:
                si = inst.sync_info
                if si is not None and len(si.on_wait) > 1:
                    waits = list(si.on_wait)
                    for w in waits[:-1]:
                        nop = mybir.InstNoOp(
                            name=nc.get_next_instruction_name(), ins=[], outs=[]
                        )
                        nop.engine = inst.engine
                        nop.sync_info = mybir.SyncInfo(on_wait=[w], on_update=[])
                        new.append(nop)
                    inst.sync_info = mybir.SyncInfo(
                        on_wait=[waits[-1]], on_update=list(si.on_update)
                    )
                    changed = True
                new.append(inst)
            if changed:
                insts[:] = new


def _ensure_ntff_hook():
    """This image's antenv lacks axon_hooks; graft a minimal one so
    trace=True can reach the NTFF profiler instead of crashing."""
    import sys
    import types

    try:
        import antenv.axon_hooks  # noqa: F401

        return
    except ImportError:
        pass
    import antenv
    from trn_agent_boot.trn_boot import _ntff_profile_via_ctypes

    mod = types.ModuleType("antenv.axon_hooks")
    _h = [None]
    mod.set_axon_ntff_profile_hook = lambda h: _h.__setitem__(0, h)
    mod.get_axon_ntff_profile_hook = lambda: _h[0]
    sys.modules["antenv.axon_hooks"] = mod
    antenv.axon_hooks = mod
    try:
        mod.set_axon_ntff_profile_hook(
            _ntff_profile_via_ctypes("/opt/axon/libaxon_pjrt.so")
        )
    except Exception:
        pass


def _build_program(mm_dt, with_qk_bias: bool):
    nc = bass.Bass()

    xt = nc.dram_tensor("xt", [C, T], mm_dt, kind="ExternalInput")
    wqkvt = nc.dram_tensor("wqkvt", [C, 3 * 512], mm_dt, kind="ExternalInput")
    woutt = nc.dram_tensor("woutt", [512, C], mm_dt, kind="ExternalInput")
    cos_d = nc.dram_tensor("cos", [128, T], _F32, kind="ExternalInput")
    sin_d = nc.dram_tensor("sin", [128, T], _F32, kind="ExternalInput")
    masks_d = nc.dram_tensor("masks", [4, 128, 512], _F32, kind="ExternalInput")
    ones_d = nc.dram_tensor("ones", [128, 128], mm_dt, kind="ExternalInput")
    bias_d = nc.dram_tensor("bias_qk", [128, 8], _F32, kind="ExternalInput")
    part = nc.dram_tensor("part", [T, C], _F32, kind="ExternalOutput")

    scale = 1.0 / math.sqrt(D)

    with TileContext(nc) as tc:
        with (
            tc.tile_pool(name="consts", bufs=1) as cpool,
            tc.tile_pool(name="attn", bufs=1) as apool,
            tc.tile_pool(name="dram", bufs=1, space="DRAM") as dpool,
        ):
            ones_sb = cpool.tile([128, 128], mm_dt, tag="ones", name="ones_sb")
            nc.sync.dma_start(ones_sb[:], ones_d[:])
            if with_qk_bias:
                bias_sb = cpool.tile([128, 8], _F32, tag="bias", name="bias_sb")
                nc.sync.dma_start(bias_sb[:], bias_d[:])

            # m-tile order in wqkvt / qk_d: qE0 qO0 qE1 qO1 kE0 kO0 kE1 kO1
            # (pair p: even dims of heads 2p,2p+1 / odd dims), then v 512.
            qk_d = [
                dpool.tile([128, T], _F32, tag=f"qk{m}", name=f"qk{m}_d")
                for m in range(8)
            ]
            v_d = [
                dpool.tile([128, 512], mm_dt, tag=f"v{t}", name=f"v{t}_d")
                for t in range(TT)
            ]
            attn = [
                apool.tile([128, T], mm_dt, tag=f"attn{r}", name=f"attn{r}_sb")
                for r in range(4)
            ]

            # ---- Phase 1: qkv projection ----
            with (
                tc.tile_pool(name="wqk", bufs=16) as wqk_pool,
                tc.tile_pool(name="wv", bufs=16) as wv_pool,
                tc.tile_pool(name="xch", bufs=16) as x_pool,
                tc.tile_pool(name="p1ps", bufs=4, space="PSUM") as p1_psum,
                tc.tile_pool(name="p1st", bufs=3) as p1_stage,
            ):
                wqk = []
                wv = []
                for k in range(KT):
                    wk = wqk_pool.tile(
                        [128, 1024], mm_dt, tag="wqk", bufs=16, name=f"wqk{k}"
                    )
                    nc.sync.dma_start(wk[:], wqkvt[k * 128 : (k + 1) * 128, 0:1024])
                    wqk.append(wk)
                    wvk = wv_pool.tile(
                        [128, 512], mm_dt, tag="wv", bufs=16, name=f"wv{k}"
                    )
                    nc.sync.dma_start(
                        wvk[:], wqkvt[k * 128 : (k + 1) * 128, 1024:1536]
                    )
                    wv.append(wvk)

                for n in range(TCH):
                    xts = []
                    for k in range(KT):
                        xk = x_pool.tile(
                            [128, 512], mm_dt, tag="x", bufs=16, name=f"x{n}_{k}"
                        )
                        nc.sync.dma_start(
                            xk[:],
                            xt[k * 128 : (k + 1) * 128, n * 512 : (n + 1) * 512],
                        )
                        xts.append(xk)
                    for m in range(8):
                        ps = p1_psum.tile(
                            [128, 512], _F32, tag="ps", name=f"qkps{n}_{m}"
                        )
                        for k in range(KT):
                            nc.tensor.matmul(
                                ps[:],
                                wqk[k][:, m * 128 : (m + 1) * 128],
                                xts[k][:],
                                start=(k == 0),
                                stop=(k == KT - 1),
                            )
                        st = p1_stage.tile(
                            [128, 512], _F32, tag="st", name=f"qkst{n}_{m}"
                        )
                        if with_qk_bias:
                            nc.vector.tensor_scalar_add(
                                st[:], ps[:], bias_sb[:, m : m + 1]
                            )
                        else:
                            nc.scalar.copy(st[:], ps[:])
                        nc.sync.dma_start(
                            qk_d[m][:, n * 512 : (n + 1) * 512], st[:]
                        )
                    for t4 in range(4):
                        tt = n * 4 + t4
                        ps = p1_psum.tile([128, 512], _F32, tag="ps", name=f"vps{tt}")
                        for k in range(KT):
                            nc.tensor.matmul(
                                ps[:],
                                xts[k][:, t4 * 128 : (t4 + 1) * 128],
                                wv[k][:],
                                start=(k == 0),
                                stop=(k == KT - 1),
                            )
                        st = p1_stage.tile([128, 512], mm_dt, tag="stv", name=f"vst{tt}")
                        nc.scalar.copy(st[:], ps[:])
                        nc.sync.dma_start(v_d[tt][:], st[:])

            # ---- Phase 2: attention, one head pair at a time ----
            with (
                tc.tile_pool(name="trig", bufs=1) as trig_pool,
                tc.tile_pool(name="qk_sb", bufs=1) as qk_sb_pool,
                tc.tile_pool(name="rot", bufs=1) as rot_pool,
                tc.tile_pool(name="vt", bufs=16) as vt_pool,
                tc.tile_pool(name="tmp", bufs=2) as tmp_pool,
                tc.tile_pool(name="exp", bufs=3) as exp_pool,
                tc.tile_pool(name="sums", bufs=2) as sums_pool,
                tc.tile_pool(name="nrm", bufs=4) as nrm_pool,
                tc.tile_pool(name="sps", bufs=2, space="PSUM") as s_psum,
                tc.tile_pool(name="ops", bufs=2, space="PSUM") as o_psum,
                tc.tile_pool(name="rps", bufs=2, space="PSUM") as r_psum,
                tc.tile_pool(name="bps", bufs=2, space="PSUM") as b_psum,
            ):
                cos_sb = trig_pool.tile([128, T], _F32, tag="cos", name="cos_sb")
                sin_sb = trig_pool.tile([128, T], _F32, tag="sin", name="sin_sb")
                nc.sync.dma_start(cos_sb[:], cos_d[:])
                nc.sync.dma_start(sin_sb[:], sin_d[:])
                mask_sb = []
                for p in range(4):
                    msk = trig_pool.tile(
                        [128, 512], _F32, tag=f"mask{p}", name=f"mask{p}_sb"
                    )
                    nc.sync.dma_start(msk[:], masks_d[p])
                    mask_sb.append(msk)

                for pr in range(2):  # head pair
                    qE = qk_sb_pool.tile([128, T], _F32, tag="qE", name=f"qE{pr}")
                    qO = qk_sb_pool.tile([128, T], _F32, tag="qO", name=f"qO{pr}")
                    kE = qk_sb_pool.tile([128, T], _F32, tag="kE", name=f"kE{pr}")
                    kO = qk_sb_pool.tile([128, T], _F32, tag="kO", name=f"kO{pr}")
                    nc.sync.dma_start(qE[:], qk_d[2 * pr][:])
                    nc.sync.dma_start(qO[:], qk_d[2 * pr + 1][:])
                    nc.sync.dma_start(kE[:], qk_d[4 + 2 * pr][:])
                    nc.sync.dma_start(kO[:], qk_d[4 + 2 * pr + 1][:])

                    # RoPE halves: P1 = E*cos - O*sin, P2 = E*sin + O*cos
                    qP1 = rot_pool.tile([128, T], mm_dt, tag="qP1", name=f"qP1_{pr}")
                    qP2 = rot_pool.tile([128, T], mm_dt, tag="qP2", name=f"qP2_{pr}")
                    kP1 = rot_pool.tile([128, T], mm_dt, tag="kP1", name=f"kP1_{pr}")
                    kP2 = rot_pool.tile([128, T], mm_dt, tag="kP2", name=f"kP2_{pr}")
                    for ei, oi, p1, p2 in (
                        (qE, qO, qP1, qP2),
                        (kE, kO, kP1, kP2),
                    ):
                        tm = tmp_pool.tile([128, T], _F32, tag="tmp", name="ropetmp")
                        nc.vector.tensor_mul(p1[:], ei[:], cos_sb[:])
                        nc.vector.tensor_mul(tm[:], oi[:], sin_sb[:])
                        nc.vector.tensor_sub(p1[:], p1[:], tm[:])
                        tm2 = tmp_pool.tile([128, T], _F32, tag="tmp", name="ropetmp2")
                        nc.vector.tensor_mul(p2[:], ei[:], sin_sb[:])
                        nc.vector.tensor_mul(tm2[:], oi[:], cos_sb[:])
                        nc.vector.tensor_add(p2[:], p2[:], tm2[:])

                    for sub in range(2):  # head within pair
                        h = 2 * pr + sub
                        roff = 64 * sub
                        vts = []
                        for t in range(TT):
                            vt = vt_pool.tile(
                                [128, 128], mm_dt, tag="vt", bufs=16,
                                name=f"vt{h}_{t}",
                            )
                            nc.sync.dma_start(
                                vt[:], v_d[t][:, h * 128 : (h + 1) * 128]
                            )
                            vts.append(vt)

                        for ic in range(TCH):
                            njt = 4 * ic + 4
                            ops = o_psum.tile(
                                [128, 512], _F32, tag="ops", name=f"ops{h}_{ic}"
                            )
                            sacc = sums_pool.tile(
                                [128, 512], mm_dt, tag="sacc", bufs=2,
                                name=f"sacc{h}_{ic}",
                            )
                            for jj in range(njt):
                                sps = s_psum.tile(
                                    [128, 512], _F32, tag="sps",
                                    name=f"sps{h}_{ic}_{jj}",
                                )
                                nc.tensor.matmul(
                                    sps[:],
                                    kP1[roff : roff + 64,
                                           jj * 128 : (jj + 1) * 128],
                                    qP1[roff : roff + 64,
                                           ic * 512 : (ic + 1) * 512],
                                    start=True,
                                    stop=False,
                                )
                                nc.tensor.matmul(
                                    sps[:],
                                    kP2[roff : roff + 64,
                                           jj * 128 : (jj + 1) * 128],
                                    qP2[roff : roff + 64,
                                           ic * 512 : (ic + 1) * 512],
                                    start=False,
                                    stop=True,
                                )
                                ex = exp_pool.tile(
                                    [128, 512], mm_dt, tag="exp", bufs=3,
                                    name=f"ex{h}_{ic}_{jj}",
                                )
                                nc.scalar.activation(
                                    ex[:], sps[:],
                                    mybir.ActivationFunctionType.Exp,
                                    scale=scale,
                                )
                                p = jj - 4 * ic
                                if p >= 0:
                                    exm = exp_pool.tile(
                                        [128, 512], mm_dt, tag="expm", bufs=3,
                                        name=f"exm{h}_{ic}_{jj}",
                                    )
                                    nc.vector.tensor_mul(
                                        exm[:], ex[:], mask_sb[p][:]
                                    )
                                    use = exm
                                else:
                                    use = ex
                                if jj == 0:
                                    nc.vector.tensor_copy(sacc[:], use[:])
                                else:
                                    nc.vector.tensor_add(sacc[:], sacc[:], use[:])
                                nc.tensor.matmul(
                                    ops[:],
                                    vts[jj][:],
                                    use[:],
                                    start=(jj == 0),
                                    stop=(jj == njt - 1),
                                )
                            rps = r_psum.tile(
                                [1, 512], _F32, tag="rps", name=f"rps{h}_{ic}"
                            )
                            nc.tensor.matmul(
                                rps[:], ones_sb[:, 0:1], sacc[:],
                                start=True, stop=True,
                            )
                            rc32 = nrm_pool.tile(
                                [1, 512], _F32, tag="rc32", bufs=2,
                                name=f"rc32_{h}_{ic}",
                            )
                            nc.vector.reciprocal(rc32[:], rps[:])
                            rc = nrm_pool.tile(
                                [1, 512], mm_dt, tag="rc", bufs=2, name=f"rc{h}_{ic}"
                            )
                            nc.scalar.copy(rc[:], rc32[:])
                            bps = b_psum.tile(
                                [128, 512], _F32, tag="bps", name=f"bps{h}_{ic}"
                            )
                            nc.tensor.matmul(
                                bps[:], ones_sb[0:1, :], rc[:],
                                start=True, stop=True,
                            )
                            bcs = nrm_pool.tile(
                                [128, 512], _F32, tag="bcs", bufs=2,
                                name=f"bcs{h}_{ic}",
                            )
                            nc.scalar.copy(bcs[:], bps[:])
                            nc.vector.tensor_mul(
                                attn[h][:, ic * 512 : (ic + 1) * 512],
                                ops[:], bcs[:],
                            )

            # ---- Phase 3: partial out-projection ----
            with (
                tc.tile_pool(name="wout", bufs=4) as wout_pool,
                tc.tile_pool(name="p3ps", bufs=4, space="PSUM") as p3_psum,
                tc.tile_pool(name="p3st", bufs=4) as p3_stage,
            ):
                wout_sb = []
                for r in range(4):
                    w = wout_pool.tile(
                        [128, C], mm_dt, tag="wout", bufs=4, name=f"wout{r}"
                    )
                    nc.sync.dma_start(w[:], woutt[r * 128 : (r + 1) * 128, :])
                    wout_sb.append(w)
                for t in range(TT):
                    for jo in range(TCH):
                        ps = p3_psum.tile(
                            [128, 512], _F32, tag="ps", name=f"ops3_{t}_{jo}"
                        )
                        for r in range(4):
                            nc.tensor.matmul(
                                ps[:],
                                attn[r][:, t * 128 : (t + 1) * 128],
                                wout_sb[r][:, jo * 512 : (jo + 1) * 512],
                                start=(r == 0),
                                stop=(r == 3),
                            )
                        st = p3_stage.tile(
                            [128, 512], _F32, tag="st", name=f"ost{t}_{jo}"
                        )
                        nc.scalar.copy(st[:], ps[:])
                        nc.sync.dma_start(
                            part[t * 128 : (t + 1) * 128,
                                 jo * 512 : (jo + 1) * 512],
                            st[:],
                        )

    _split_multiwait(nc)
    return nc


def _host_inputs(x, W_qkv, b_qkv, W_out):
    """Per-core input maps (host-side shard + transpose + tables)."""
    even = np.arange(0, D, 2)
    odd = np.arange(1, D, 2)

    inv_freq = 1.0 / (10000.0 ** (np.arange(0, D, 2, dtype=np.float64) / D))
    tpos = np.arange(T, dtype=np.float64)
    freqs = tpos[None, :] * inv_freq[:, None]  # [64, T]
    cos64 = np.cos(freqs)
    sin64 = np.sin(freqs)
    # duplicated per head pair: rows 0:64 head A, 64:128 head B
    cos = np.concatenate([cos64, cos64], axis=0).astype(np.float32)
    sin = np.concatenate([sin64, sin64], axis=0).astype(np.float32)

    masks = np.zeros((4, 128, 512), dtype=np.float32)
    jjj = np.arange(128)[:, None]
    iii = np.arange(512)[None, :]
    for p in range(4):
        masks[p] = (jjj + 128 * p <= iii).astype(np.float32)

    ones = np.ones((128, 128), dtype=np.float32)

    in_maps = []
    for c in range(NC):
        b, g = divmod(c, 4)
        # m-tiles: qE0 qO0 qE1 qO1 (pair-local heads (0,1),(2,3))
        def pair_rows(base):
            rows = []
            for pr in range(2):
                hA = base + 512 * g + 128 * (2 * pr)
                hB = base + 512 * g + 128 * (2 * pr + 1)
                rows.append(np.concatenate([hA + even, hB + even]))  # E tile
                rows.append(np.concatenate([hA + odd, hB + odd]))  # O tile
            return rows

        qk_rows = np.concatenate(pair_rows(0) + pair_rows(C))  # [2048]
        vrows = 2 * C + 512 * g + np.arange(512)
        rows = np.concatenate([qk_rows, vrows])
        wqkvt = np.ascontiguousarray(W_qkv[rows].T)  # [C, 1536]
        xt = np.ascontiguousarray(x[b].T)  # [C, T]
        woutt = np.ascontiguousarray(W_out[:, 512 * g : 512 * (g + 1)].T)
        bias_qk = np.ascontiguousarray(b_qkv[qk_rows].reshape(8, 128).T)
        in_maps.append(
            {
                "xt": xt,
                "wqkvt": wqkvt,
                "woutt": woutt,
                "cos": cos,
                "sin": sin,
                "masks": masks,
                "ones": ones,
                "bias_qk": bias_qk,
            }
        )
    return in_maps


def kernel(x, W_qkv, b_qkv, W_out, b_out, mm_dt="float32r", trace=False):
    x = np.asarray(x, dtype=np.float32)
    W_qkv = np.asarray(W_qkv, dtype=np.float32)
    b_qkv = np.asarray(b_qkv, dtype=np.float32)
    W_out = np.asarray(W_out, dtype=np.float32)
    b_out = np.asarray(b_out, dtype=np.float32)

    mm = mybir.dt.float32r if mm_dt == "float32r" else mybir.dt.float32
    with_qk_bias = bool(np.any(b_qkv[: 2 * C]))
    nc = _build_program(mm, with_qk_bias)
    in_maps = _host_inputs(x, W_qkv, b_qkv, W_out)

    kwargs = {}
    if trace:
        _ensure_ntff_hook()
        kwargs = dict(trace=True, trace_cores=[0])
    res = run_bass_kernel_spmd(nc, in_maps, core_ids=list(range(NC)), **kwargs)

    # host "all-reduce": sum the 4 partials per batch, add biases (the v-bias
    # passes through softmax exactly: attn rows sum to 1)
    corr = b_out + W_out @ b_qkv[2 * C :]
    out = np.empty((B, T, C), dtype=np.float32)
    for b in range(B):
        acc = res.results[4 * b]["part"].astype(np.float32)
        for c in range(4 * b + 1, 4 * b + 4):
            acc = acc + res.results[c]["part"]
        out[b] = acc + corr[None, :]

    if trace:
        return out, res.exec_time_ns
    return out


# revision 8
# speedup vs baseline: 1.0938x; 1.0938x over previous
"""Trainium2 Bass kernel: causal multi-head attention with RoPE + out-proj.

Problem shapes (hardcoded): x [2, 2048, 2048], W_qkv [6144, 2048],
b_qkv [6144], W_out [2048, 2048], b_out [2048]. H=16 heads, D=128.

Sharding over 8 NeuronCores: core c handles batch b = c//4 and head group
g = c%4 (4 heads). Each core computes its heads' attention output and a
partial out-projection over its 512 columns of the contraction; the host
sums the 4 partials per batch (the "all-reduce") and adds biases.

Device-side layout choices (matmuls are [K,M]@[K,N], N=512 free dim):
- host pre-transposes x and the weight shards so contractions land on
  partitions with contiguous DMA reads
- q/k are produced transposed ([dim, token]) in head PAIRS: one 128-row
  tile holds the even dims of both heads, another the odd dims, so RoPE
  is full-width 128-partition DVE math with same-start-partition
  operands (walrus's TensorTensor verifier rejects mixed partition
  offsets); the scores contraction then splits into two K=64 matmuls
  (first-half dims + second-half dims) accumulating in PSUM
- v is produced in natural layout ([token, dim]) so it can be the
  stationary operand of the attention@V matmul
- scores are computed transposed (keys on partitions, queries free);
  softmax skips the max-subtraction (scores here are ~N(0,1), exp can't
  overflow in fp32) so exp needs no per-row bias, the causal mask is a
  multiplicative 0/1 mask after exp, and the denominator comes from a
  ones-vector matmul over the accumulated exp tiles
"""

import math

import numpy as np

import concourse.bass as bass
import concourse.mybir as mybir
from concourse.bass_utils import run_bass_kernel_spmd
from concourse.tile import TileContext
from concourse.vector_clock import ScopedClock

B, T, C = 2, 2048, 2048
H, D = 16, 128
G = 4  # heads per core
NC = 8  # cores
KT = C // 128  # 16 contraction tiles
TT = T // 128  # 16 token tiles
TCH = T // 512  # 4 token chunks

_F32 = mybir.dt.float32


def _install_drain_patch():
    """Walrus in this env rejects >1 sync wait on one CTRL (Drain) inst.

    Tile's tail drain attaches one wait per outstanding logical proc to a
    single Drain; split them across single-wait NOPs on SP instead.
    """

    def _drain_and_barrier(self, tick_clock, wait_clock):
        probe = self.nc.sync.nop(nofuse=True)
        wait_clock.add_sem_waits(
            probe.ins, ScopedClock({None: tick_clock.global_clock})
        )
        si = probe.ins.sync_info
        if si is not None and len(si.on_wait) > 1:
            waits = list(si.on_wait)
            probe.ins.sync_info = mybir.SyncInfo(
                on_wait=waits[:1], on_update=list(si.on_update)
            )
            for w in waits[1:]:
                extra = self.nc.sync.nop(nofuse=True)
                extra.ins.sync_info = mybir.SyncInfo(on_wait=[w], on_update=[])
        self.nc.sync.drain()
        self.nc.all_engine_barrier()
        popped = self.nc._tile_sem_poison_stack.pop()
        assert popped is self._sem_poison
        self.nc.clear_and_free_semaphores(list(self.sems.allocated().values()))
        self.nc.all_engine_barrier()

    TileContext._drain_and_barrier = _drain_and_barrier


_install_drain_patch()


def _split_multiwait(nc):
    """Walrus here allows only one sync wait per instruction: move extras
    onto single-wait NOPs inserted just before, on the same engine."""
    for fn in nc.m.functions:
        for bb in fn.blocks:
            insts = bb.instructions
            new = []
            changed = False
            for inst in insts:
                si = inst.sync_info
                if si is not None and len(si.on_wait) > 1:
                    waits = list(si.on_wait)
                    for w in waits[:-1]:
                        nop = mybir.InstNoOp(
                            name=nc.get_next_instruction_name(), ins=[], outs=[]
                        )
                        nop.engine = inst.engine
                        nop.sync_info = mybir.SyncInfo(on_wait=[w], on_update=[])
                        new.append(nop)
                    inst.sync_info = mybir.SyncInfo(
                        on_wait=[waits[-1]], on_update=list(si.on_update)
                    )
                    changed = True
                new.append(inst)
            if changed:
                insts[:] = new


def _ensure_ntff_hook():
    """This image's antenv lacks axon_hooks; graft a minimal one so
    trace=True can reach the NTFF profiler instead of crashing."""
    import sys
    import types

    try:
        import antenv.axon_hooks  # noqa: F401

        return
    except ImportError:
        pass
    import antenv
    from trn_agent_boot.trn_boot import _ntff_profile_via_ctypes

    mod = types.ModuleType("antenv.axon_hooks")
    _h = [None]
    mod.set_axon_ntff_profile_hook = lambda h: _h.__setitem__(0, h)
    mod.get_axon_ntff_profile_hook = lambda: _h[0]
    sys.modules["antenv.axon_hooks"] = mod
    antenv.axon_hooks = mod
    try:
        mod.set_axon_ntff_profile_hook(
            _ntff_profile_via_ctypes("/opt/axon/libaxon_pjrt.so")
        )
    except Exception:
        pass


def _build_program(mm_dt, with_qk_bias: bool):
    nc = bass.Bass()

    xt = nc.dram_tensor("xt", [C, T], mm_dt, kind="ExternalInput")
    wqkvt = nc.dram_tensor("wqkvt", [C, 3 * 512], mm_dt, kind="ExternalInput")
    woutt = nc.dram_tensor("woutt", [512, C], mm_dt, kind="ExternalInput")
    cos_d = nc.dram_tensor("cos", [128, T], _F32, kind="ExternalInput")
    sin_d = nc.dram_tensor("sin", [128, T], _F32, kind="ExternalInput")
    masks_d = nc.dram_tensor("masks", [4, 128, 512], _F32, kind="ExternalInput")
    ones_d = nc.dram_tensor("ones", [128, 128], mm_dt, kind="ExternalInput")
    bias_d = nc.dram_tensor("bias_qk", [128, 8], _F32, kind="ExternalInput")
    part = nc.dram_tensor("part", [T, C], _F32, kind="ExternalOutput")

    scale = 1.0 / math.sqrt(D)

    with TileContext(nc) as tc:
        with (
            tc.tile_pool(name="consts", bufs=1) as cpool,
            tc.tile_pool(name="attn", bufs=1) as apool,
            tc.tile_pool(name="dram", bufs=1, space="DRAM") as dpool,
        ):
            ones_sb = cpool.tile([128, 128], mm_dt, tag="ones", name="ones_sb")
            nc.sync.dma_start(ones_sb[:], ones_d[:])
            if with_qk_bias:
                bias_sb = cpool.tile([128, 8], _F32, tag="bias", name="bias_sb")
                nc.sync.dma_start(bias_sb[:], bias_d[:])

            # m-tile order in wqkvt / qk_d: qE0 qO0 qE1 qO1 kE0 kO0 kE1 kO1
            # (pair p: even dims of heads 2p,2p+1 / odd dims), then v 512.
            qk_d = [
                dpool.tile([128, T], _F32, tag=f"qk{m}", name=f"qk{m}_d")
                for m in range(8)
            ]
            v_d = [
                dpool.tile([128, 512], mm_dt, tag=f"v{t}", name=f"v{t}_d")
                for t in range(TT)
            ]
            attn = [
                apool.tile([128, T], mm_dt, tag=f"attn{r}", name=f"attn{r}_sb")
                for r in range(4)
            ]

            # ---- Phase 1: qkv projection ----
            with (
                tc.tile_pool(name="wqk", bufs=16) as wqk_pool,
                tc.tile_pool(name="wv", bufs=16) as wv_pool,
                tc.tile_pool(name="xch", bufs=16) as x_pool,
                tc.tile_pool(name="p1ps", bufs=4, space="PSUM") as p1_psum,
                tc.tile_pool(name="p1st", bufs=3) as p1_stage,
            ):
                wqk = []
                wv = []
                for k in range(KT):
                    wk = wqk_pool.tile(
                        [128, 1024], mm_dt, tag="wqk", bufs=16, name=f"wqk{k}"
                    )
                    nc.sync.dma_start(wk[:], wqkvt[k * 128 : (k + 1) * 128, 0:1024])
                    wqk.append(wk)
                    wvk = wv_pool.tile(
                        [128, 512], mm_dt, tag="wv", bufs=16, name=f"wv{k}"
                    )
                    nc.sync.dma_start(
                        wvk[:], wqkvt[k * 128 : (k + 1) * 128, 1024:1536]
                    )
                    wv.append(wvk)

                for n in range(TCH):
                    xts = []
                    for k in range(KT):
                        xk = x_pool.tile(
                            [128, 512], mm_dt, tag="x", bufs=16, name=f"x{n}_{k}"
                        )
                        nc.sync.dma_start(
                            xk[:],
                            xt[k * 128 : (k + 1) * 128, n * 512 : (n + 1) * 512],
                        )
                        xts.append(xk)
                    for m in range(8):
                        ps = p1_psum.tile(
                            [128, 512], _F32, tag="ps", name=f"qkps{n}_{m}"
                        )
                        for k in range(KT):
                            nc.tensor.matmul(
                                ps[:],
                                wqk[k][:, m * 128 : (m + 1) * 128],
                                xts[k][:],
                                start=(k == 0),
                                stop=(k == KT - 1),
                            )
                        st = p1_stage.tile(
                            [128, 512], _F32, tag="st", name=f"qkst{n}_{m}"
                        )
                        if with_qk_bias:
                            nc.vector.tensor_scalar_add(
                                st[:], ps[:], bias_sb[:, m : m + 1]
                            )
                        else:
                            nc.scalar.copy(st[:], ps[:])
                        nc.sync.dma_start(
                            qk_d[m][:, n * 512 : (n + 1) * 512], st[:]
                        )
                    for t4 in range(4):
                        tt = n * 4 + t4
                        ps = p1_psum.tile([128, 512], _F32, tag="ps", name=f"vps{tt}")
                        for k in range(KT):
                            nc.tensor.matmul(
                                ps[:],
                                xts[k][:, t4 * 128 : (t4 + 1) * 128],
                                wv[k][:],
                                start=(k == 0),
                                stop=(k == KT - 1),
                            )
                        st = p1_stage.tile([128, 512], mm_dt, tag="stv", name=f"vst{tt}")
                        nc.scalar.copy(st[:], ps[:])
                        nc.sync.dma_start(v_d[tt][:], st[:])

            # ---- Phase 2: attention, one head pair at a time ----
            with (
                tc.tile_pool(name="trig", bufs=1) as trig_pool,
                tc.tile_pool(name="eo", bufs=3) as eo_pool,
                tc.tile_pool(name="rot", bufs=1) as rot_pool,
                tc.tile_pool(name="vt", bufs=16) as vt_pool,
                tc.tile_pool(name="tmp", bufs=4) as tmp_pool,
                tc.tile_pool(name="exp", bufs=4) as exp_pool,
                tc.tile_pool(name="sums", bufs=2) as sums_pool,
                tc.tile_pool(name="nrm", bufs=4) as nrm_pool,
                tc.tile_pool(name="sps", bufs=3, space="PSUM") as s_psum,
                tc.tile_pool(name="ops", bufs=2, space="PSUM") as o_psum,
                tc.tile_pool(name="rps", bufs=1, space="PSUM") as r_psum,
                tc.tile_pool(name="bps", bufs=1, space="PSUM") as b_psum,
            ):
                cos_sb = trig_pool.tile([128, T], _F32, tag="cos", name="cos_sb")
                sin_sb = trig_pool.tile([128, T], _F32, tag="sin", name="sin_sb")
                nc.sync.dma_start(cos_sb[:], cos_d[:])
                nc.sync.dma_start(sin_sb[:], sin_d[:])
                mask_sb = []
                for p in range(4):
                    msk = trig_pool.tile(
                        [128, 512], _F32, tag=f"mask{p}", name=f"mask{p}_sb"
                    )
                    nc.sync.dma_start(msk[:], masks_d[p])
                    mask_sb.append(msk)

                for pr in range(2):  # head pair
                    # per-head rotated q/k [dim, token]; RoPE reads the
                    # pair-layout E/O chunks and writes across partition
                    # offsets (outputs may start on a different partition
                    # than inputs; only inputs must agree)
                    rq = [
                        rot_pool.tile(
                            [128, T], mm_dt, tag=f"rq{s}", bufs=2,
                            name=f"rq{pr}_{s}",
                        )
                        for s in range(2)
                    ]
                    rk = [
                        rot_pool.tile(
                            [128, T], mm_dt, tag=f"rk{s}", bufs=2,
                            name=f"rk{pr}_{s}",
                        )
                        for s in range(2)
                    ]
                    for src_q, dsts in ((True, rq), (False, rk)):
                        mE = (0 if src_q else 4) + 2 * pr
                        for ch in range(TCH):
                            cs = slice(ch * 512, (ch + 1) * 512)
                            E = eo_pool.tile(
                                [128, 512], _F32, tag="E", bufs=3,
                                name=f"E{pr}_{src_q}_{ch}",
                            )
                            O = eo_pool.tile(
                                [128, 512], _F32, tag="O", bufs=3,
                                name=f"O{pr}_{src_q}_{ch}",
                            )
                            nc.sync.dma_start(E[:], qk_d[mE][:, cs])
                            nc.sync.dma_start(O[:], qk_d[mE + 1][:, cs])
                            for s in range(2):
                                ro = 64 * s
                                rs = slice(ro, ro + 64)
                                dst = dsts[s]
                                t1 = tmp_pool.tile(
                                    [128, 512], _F32, tag="tmp", bufs=4,
                                    name="ropet1",
                                )
                                t2 = tmp_pool.tile(
                                    [128, 512], _F32, tag="tmp", bufs=4,
                                    name="ropet2",
                                )
                                nc.vector.tensor_mul(
                                    t1[rs, :], E[rs, :], cos_sb[rs, cs]
                                )
                                nc.vector.tensor_mul(
                                    t2[rs, :], O[rs, :], sin_sb[rs, cs]
                                )
                                nc.vector.tensor_sub(
                                    dst[0:64, cs], t1[rs, :], t2[rs, :]
                                )
                                t3 = tmp_pool.tile(
                                    [128, 512], _F32, tag="tmp", bufs=4,
                                    name="ropet3",
                                )
                                t4 = tmp_pool.tile(
                                    [128, 512], _F32, tag="tmp", bufs=4,
                                    name="ropet4",
                                )
                                nc.vector.tensor_mul(
                                    t3[rs, :], E[rs, :], sin_sb[rs, cs]
                                )
                                nc.vector.tensor_mul(
                                    t4[rs, :], O[rs, :], cos_sb[rs, cs]
                                )
                                nc.vector.tensor_add(
                                    dst[64:128, cs], t3[rs, :], t4[rs, :]
                                )

                    for sub in range(2):  # head within pair
                        h = 2 * pr + sub
                        vts = []
                        for t in range(TT):
                            vt = vt_pool.tile(
                                [128, 128], mm_dt, tag="vt", bufs=16,
                                name=f"vt{h}_{t}",
                            )
                            nc.sync.dma_start(
                                vt[:], v_d[t][:, h * 128 : (h + 1) * 128]
                            )
                            vts.append(vt)

                        for ic in range(TCH):
                            njt = 4 * ic + 4
                            ops = o_psum.tile(
                                [128, 512], _F32, tag="ops", name=f"ops{h}_{ic}"
                            )
                            sacc = sums_pool.tile(
                                [128, 512], mm_dt, tag="sacc", bufs=2,
                                name=f"sacc{h}_{ic}",
                            )
                            for jj in range(njt):
                                sps = s_psum.tile(
                                    [128, 512], _F32, tag="sps",
                                    name=f"sps{h}_{ic}_{jj}",
                                )
                                nc.tensor.matmul(
                                    sps[:],
                                    rk[sub][:, jj * 128 : (jj + 1) * 128],
                                    rq[sub][:, ic * 512 : (ic + 1) * 512],
                                    start=True,
                                    stop=True,
                                )
                                ex = exp_pool.tile(
                                    [128, 512], mm_dt, tag="exp", bufs=4,
                                    name=f"ex{h}_{ic}_{jj}",
                                )
                                nc.scalar.activation(
                                    ex[:], sps[:],
                                    mybir.ActivationFunctionType.Exp,
                                    scale=scale,
                                )
                                p = jj - 4 * ic
                                if p >= 0:
                                    exm = exp_pool.tile(
                                        [128, 512], mm_dt, tag="expm", bufs=4,
                                        name=f"exm{h}_{ic}_{jj}",
                                    )
                                    nc.vector.tensor_mul(
                                        exm[:], ex[:], mask_sb[p][:]
                                    )
                                    use = exm
                                else:
                                    use = ex
                                if jj == 0:
                                    nc.vector.tensor_copy(sacc[:], use[:])
                                else:
                                    nc.vector.tensor_add(sacc[:], sacc[:], use[:])
                                nc.tensor.matmul(
                                    ops[:],
                                    vts[jj][:],
                                    use[:],
                                    start=(jj == 0),
                                    stop=(jj == njt - 1),
                                )
                            rps = r_psum.tile(
                                [1, 512], _F32, tag="rps", name=f"rps{h}_{ic}"
                            )
                            nc.tensor.matmul(
                                rps[:], ones_sb[:, 0:1], sacc[:],
                                start=True, stop=True,
                            )
                            rc32 = nrm_pool.tile(
                                [1, 512], _F32, tag="rc32", bufs=2,
                                name=f"rc32_{h}_{ic}",
                            )
                            nc.vector.reciprocal(rc32[:], rps[:])
                            rc = nrm_pool.tile(
                                [1, 512], mm_dt, tag="rc", bufs=2, name=f"rc{h}_{ic}"
                            )
                            nc.scalar.copy(rc[:], rc32[:])
                            bps = b_psum.tile(
                                [128, 512], _F32, tag="bps", name=f"bps{h}_{ic}"
                            )
                            nc.tensor.matmul(
                                bps[:], ones_sb[0:1, :], rc[:],
                                start=True, stop=True,
                            )
                            bcs = nrm_pool.tile(
                                [128, 512], _F32, tag="bcs", bufs=2,
                                name=f"bcs{h}_{ic}",
                            )
                            nc.scalar.copy(bcs[:], bps[:])
                            nc.vector.tensor_mul(
                                attn[h][:, ic * 512 : (ic + 1) * 512],
                                ops[:], bcs[:],
                            )

            # ---- Phase 3: partial out-projection ----
            with (
                tc.tile_pool(name="wout", bufs=4) as wout_pool,
                tc.tile_pool(name="p3ps", bufs=4, space="PSUM") as p3_psum,
                tc.tile_pool(name="p3st", bufs=4) as p3_stage,
            ):
                wout_sb = []
                for r in range(4):
                    w = wout_pool.tile(
                        [128, C], mm_dt, tag="wout", bufs=4, name=f"wout{r}"
                    )
                    nc.sync.dma_start(w[:], woutt[r * 128 : (r + 1) * 128, :])
                    wout_sb.append(w)
                for t in range(TT):
                    for jo in range(TCH):
                        ps = p3_psum.tile(
                            [128, 512], _F32, tag="ps", name=f"ops3_{t}_{jo}"
                        )
                        for r in range(4):
                            nc.tensor.matmul(
                                ps[:],
                                attn[r][:, t * 128 : (t + 1) * 128],
                                wout_sb[r][:, jo * 512 : (jo + 1) * 512],
                                start=(r == 0),
                                stop=(r == 3),
                            )
                        st = p3_stage.tile(
                            [128, 512], _F32, tag="st", name=f"ost{t}_{jo}"
                        )
                        nc.scalar.copy(st[:], ps[:])
                        nc.sync.dma_start(
                            part[t * 128 : (t + 1) * 128,
                                 jo * 512 : (jo + 1) * 512],
                            st[:],
                        )

    _split_multiwait(nc)
    return nc


def _host_inputs(x, W_qkv, b_qkv, W_out):
    """Per-core input maps (host-side shard + transpose + tables)."""
    even = np.arange(0, D, 2)
    odd = np.arange(1, D, 2)

    inv_freq = 1.0 / (10000.0 ** (np.arange(0, D, 2, dtype=np.float64) / D))
    tpos = np.arange(T, dtype=np.float64)
    freqs = tpos[None, :] * inv_freq[:, None]  # [64, T]
    cos64 = np.cos(freqs)
    sin64 = np.sin(freqs)
    # duplicated per head pair: rows 0:64 head A, 64:128 head B
    cos = np.concatenate([cos64, cos64], axis=0).astype(np.float32)
    sin = np.concatenate([sin64, sin64], axis=0).astype(np.float32)

    masks = np.zeros((4, 128, 512), dtype=np.float32)
    jjj = np.arange(128)[:, None]
    iii = np.arange(512)[None, :]
    for p in range(4):
        masks[p] = (jjj + 128 * p <= iii).astype(np.float32)

    ones = np.ones((128, 128), dtype=np.float32)

    in_maps = []
    for c in range(NC):
        b, g = divmod(c, 4)
        # m-tiles: qE0 qO0 qE1 qO1 (pair-local heads (0,1),(2,3))
        def pair_rows(base):
            rows = []
            for pr in range(2):
                hA = base + 512 * g + 128 * (2 * pr)
                hB = base + 512 * g + 128 * (2 * pr + 1)
                rows.append(np.concatenate([hA + even, hB + even]))  # E tile
                rows.append(np.concatenate([hA + odd, hB + odd]))  # O tile
            return rows

        qk_rows = np.concatenate(pair_rows(0) + pair_rows(C))  # [2048]
        vrows = 2 * C + 512 * g + np.arange(512)
        rows = np.concatenate([qk_rows, vrows])
        wqkvt = np.ascontiguousarray(W_qkv[rows].T)  # [C, 1536]
        xt = np.ascontiguousarray(x[b].T)  # [C, T]
        woutt = np.ascontiguousarray(W_out[:, 512 * g : 512 * (g + 1)].T)
        bias_qk = np.ascontiguousarray(b_qkv[qk_rows].reshape(8, 128).T)
        in_maps.append(
            {
                "xt": xt,
                "wqkvt": wqkvt,
                "woutt": woutt,
                "cos": cos,
                "sin": sin,
                "masks": masks,
                "ones": ones,
                "bias_qk": bias_qk,
            }
        )
    return in_maps


def kernel(x, W_qkv, b_qkv, W_out, b_out, mm_dt="float32r", trace=False):
    x = np.asarray(x, dtype=np.float32)
    W_qkv = np.asarray(W_qkv, dtype=np.float32)
    b_qkv = np.asarray(b_qkv, dtype=np.float32)
    W_out = np.asarray(W_out, dtype=np.float32)
    b_out = np.asarray(b_out, dtype=np.float32)

    mm = mybir.dt.float32r if mm_dt == "float32r" else mybir.dt.float32
    with_qk_bias = bool(np.any(b_qkv[: 2 * C]))
    nc = _build_program(mm, with_qk_bias)
    in_maps = _host_inputs(x, W_qkv, b_qkv, W_out)

    kwargs = {}
    if trace:
        _ensure_ntff_hook()
        kwargs = dict(trace=True, trace_cores=[0])
    res = run_bass_kernel_spmd(nc, in_maps, core_ids=list(range(NC)), **kwargs)

    # host "all-reduce": sum the 4 partials per batch, add biases (the v-bias
    # passes through softmax exactly: attn rows sum to 1)
    corr = b_out + W_out @ b_qkv[2 * C :]
    out = np.empty((B, T, C), dtype=np.float32)
    for b in range(B):
        acc = res.results[4 * b]["part"].astype(np.float32)
        for c in range(4 * b + 1, 4 * b + 4):
            acc = acc + res.results[c]["part"]
        out[b] = acc + corr[None, :]

    if trace:
        return out, res.exec_time_ns
    return out


# revision 9
# speedup vs baseline: 1.2348x; 1.1289x over previous
"""Trainium2 Bass kernel: causal multi-head attention with RoPE + out-proj.

Problem shapes (hardcoded): x [2, 2048, 2048], W_qkv [6144, 2048],
b_qkv [6144], W_out [2048, 2048], b_out [2048]. H=16 heads, D=128.

Sharding over 8 NeuronCores: core c handles batch b = c//4 and head group
g = c%4 (4 heads). Each core computes its heads' attention output and a
partial out-projection over its 512 columns of the contraction; the host
sums the 4 partials per batch (the "all-reduce") and adds biases.

Device-side layout choices (matmuls are [K,M]@[K,N], N=512 free dim):
- host pre-transposes x and the weight shards so contractions land on
  partitions with contiguous DMA reads
- q/k are produced transposed ([dim, token]) in head PAIRS: one 128-row
  tile holds the even dims of both heads, another the odd dims, so RoPE
  is full-width 128-partition DVE math with same-start-partition
  operands (walrus's TensorTensor verifier rejects mixed partition
  offsets); the scores contraction then splits into two K=64 matmuls
  (first-half dims + second-half dims) accumulating in PSUM
- v is produced in natural layout ([token, dim]) so it can be the
  stationary operand of the attention@V matmul
- scores are computed transposed (keys on partitions, queries free);
  softmax skips the max-subtraction (scores here are ~N(0,1), exp can't
  overflow in fp32) so exp needs no per-row bias, the causal mask is a
  multiplicative 0/1 mask after exp, and the denominator comes from a
  ones-vector matmul over the accumulated exp tiles
"""

import math

import numpy as np

import concourse.bass as bass
import concourse.mybir as mybir
from concourse.bass_utils import run_bass_kernel_spmd
from concourse.tile import TileContext
from concourse.vector_clock import ScopedClock

B, T, C = 2, 2048, 2048
H, D = 16, 128
G = 4  # heads per core
NC = 8  # cores
KT = C // 128  # 16 contraction tiles
TT = T // 128  # 16 token tiles
TCH = T // 512  # 4 token chunks

_F32 = mybir.dt.float32


def _install_drain_patch():
    """Walrus in this env rejects >1 sync wait on one CTRL (Drain) inst.

    Tile's tail drain attaches one wait per outstanding logical proc to a
    single Drain; split them across single-wait NOPs on SP instead.
    """

    def _drain_and_barrier(self, tick_clock, wait_clock):
        probe = self.nc.sync.nop(nofuse=True)
        wait_clock.add_sem_waits(
            probe.ins, ScopedClock({None: tick_clock.global_clock})
        )
        si = probe.ins.sync_info
        if si is not None and len(si.on_wait) > 1:
            waits = list(si.on_wait)
            probe.ins.sync_info = mybir.SyncInfo(
                on_wait=waits[:1], on_update=list(si.on_update)
            )
            for w in waits[1:]:
                extra = self.nc.sync.nop(nofuse=True)
                extra.ins.sync_info = mybir.SyncInfo(on_wait=[w], on_update=[])
        self.nc.sync.drain()
        self.nc.all_engine_barrier()
        popped = self.nc._tile_sem_poison_stack.pop()
        assert popped is self._sem_poison
        self.nc.clear_and_free_semaphores(list(self.sems.allocated().values()))
        self.nc.all_engine_barrier()

    TileContext._drain_and_barrier = _drain_and_barrier


_install_drain_patch()


def _split_multiwait(nc):
    """Walrus here allows only one sync wait per instruction: move extras
    onto single-wait NOPs inserted just before, on the same engine."""
    for fn in nc.m.functions:
        for bb in fn.blocks:
            insts = bb.instructions
            new = []
            changed = False
            for inst in insts:
                si = inst.sync_info
                if si is not None and len(si.on_wait) > 1:
                    waits = list(si.on_wait)
                    for w in waits[:-1]:
                        nop = mybir.InstNoOp(
                            name=nc.get_next_instruction_name(), ins=[], outs=[]
                        )
                        nop.engine = inst.engine
                        nop.sync_info = mybir.SyncInfo(on_wait=[w], on_update=[])
                        new.append(nop)
                    inst.sync_info = mybir.SyncInfo(
                        on_wait=[waits[-1]], on_update=list(si.on_update)
                    )
                    changed = True
                new.append(inst)
            if changed:
                insts[:] = new


def _ensure_ntff_hook():
    """This image's antenv lacks axon_hooks; graft a minimal one so
    trace=True can reach the NTFF profiler instead of crashing."""
    import sys
    import types

    try:
        import antenv.axon_hooks  # noqa: F401

        return
    except ImportError:
        pass
    import antenv
    from trn_agent_boot.trn_boot import _ntff_profile_via_ctypes

    mod = types.ModuleType("antenv.axon_hooks")
    _h = [None]
    mod.set_axon_ntff_profile_hook = lambda h: _h.__setitem__(0, h)
    mod.get_axon_ntff_profile_hook = lambda: _h[0]
    sys.modules["antenv.axon_hooks"] = mod
    antenv.axon_hooks = mod
    try:
        mod.set_axon_ntff_profile_hook(
            _ntff_profile_via_ctypes("/opt/axon/libaxon_pjrt.so")
        )
    except Exception:
        pass


def _build_program(mm_dt, with_qk_bias: bool):
    nc = bass.Bass()

    xt = nc.dram_tensor("xt", [C, T], mm_dt, kind="ExternalInput")
    wqkvt = nc.dram_tensor("wqkvt", [C, 3 * 512], mm_dt, kind="ExternalInput")
    woutt = nc.dram_tensor("woutt", [512, C], mm_dt, kind="ExternalInput")
    cos_d = nc.dram_tensor("cos", [128, T], _F32, kind="ExternalInput")
    sin_d = nc.dram_tensor("sin", [128, T], _F32, kind="ExternalInput")
    masks_d = nc.dram_tensor("masks", [4, 128, 512], _F32, kind="ExternalInput")
    ones_d = nc.dram_tensor("ones", [128, 128], mm_dt, kind="ExternalInput")
    bias_d = nc.dram_tensor("bias_qk", [128, 8], _F32, kind="ExternalInput")
    part = nc.dram_tensor("part", [T, C], _F32, kind="ExternalOutput")

    scale = 1.0 / math.sqrt(D)

    with TileContext(nc) as tc:
        with (
            tc.tile_pool(name="consts", bufs=1) as cpool,
            tc.tile_pool(name="attn", bufs=1) as apool,
            tc.tile_pool(name="dram", bufs=1, space="DRAM") as dpool,
        ):
            ones_sb = cpool.tile([128, 128], mm_dt, tag="ones", name="ones_sb")
            nc.sync.dma_start(ones_sb[:], ones_d[:])
            if with_qk_bias:
                bias_sb = cpool.tile([128, 8], _F32, tag="bias", name="bias_sb")
                nc.sync.dma_start(bias_sb[:], bias_d[:])

            # m-tile order in wqkvt / qk_d: qE0 qO0 qE1 qO1 kE0 kO0 kE1 kO1
            # (pair p: even dims of heads 2p,2p+1 / odd dims), then v 512.
            qk_d = [
                dpool.tile([128, T], _F32, tag=f"qk{m}", name=f"qk{m}_d")
                for m in range(8)
            ]
            v_d = [
                dpool.tile([128, 512], mm_dt, tag=f"v{t}", name=f"v{t}_d")
                for t in range(TT)
            ]
            attn = [
                apool.tile([128, T], mm_dt, tag=f"attn{r}", name=f"attn{r}_sb")
                for r in range(4)
            ]

            # ---- Phase 1: qkv projection ----
            with (
                tc.tile_pool(name="wqk", bufs=16) as wqk_pool,
                tc.tile_pool(name="wv", bufs=16) as wv_pool,
                tc.tile_pool(name="xch", bufs=16) as x_pool,
                tc.tile_pool(name="p1ps", bufs=4, space="PSUM") as p1_psum,
                tc.tile_pool(name="p1st", bufs=3) as p1_stage,
            ):
                wqk = []
                wv = []
                for k in range(KT):
                    wk = wqk_pool.tile(
                        [128, 1024], mm_dt, tag="wqk", bufs=16, name=f"wqk{k}"
                    )
                    nc.sync.dma_start(wk[:], wqkvt[k * 128 : (k + 1) * 128, 0:1024])
                    wqk.append(wk)
                    wvk = wv_pool.tile(
                        [128, 512], mm_dt, tag="wv", bufs=16, name=f"wv{k}"
                    )
                    nc.sync.dma_start(
                        wvk[:], wqkvt[k * 128 : (k + 1) * 128, 1024:1536]
                    )
                    wv.append(wvk)

                for n in range(TCH):
                    xts = []
                    for k in range(KT):
                        xk = x_pool.tile(
                            [128, 512], mm_dt, tag="x", bufs=16, name=f"x{n}_{k}"
                        )
                        nc.sync.dma_start(
                            xk[:],
                            xt[k * 128 : (k + 1) * 128, n * 512 : (n + 1) * 512],
                        )
                        xts.append(xk)
                    for m in range(8):
                        ps = p1_psum.tile(
                            [128, 512], _F32, tag="ps", name=f"qkps{n}_{m}"
                        )
                        for k in range(KT):
                            nc.tensor.matmul(
                                ps[:],
                                wqk[k][:, m * 128 : (m + 1) * 128],
                                xts[k][:],
                                start=(k == 0),
                                stop=(k == KT - 1),
                            )
                        st = p1_stage.tile(
                            [128, 512], _F32, tag="st", name=f"qkst{n}_{m}"
                        )
                        if with_qk_bias:
                            nc.vector.tensor_scalar_add(
                                st[:], ps[:], bias_sb[:, m : m + 1]
                            )
                        else:
                            nc.scalar.copy(st[:], ps[:])
                        nc.sync.dma_start(
                            qk_d[m][:, n * 512 : (n + 1) * 512], st[:]
                        )
                    for t4 in range(4):
                        tt = n * 4 + t4
                        ps = p1_psum.tile([128, 512], _F32, tag="ps", name=f"vps{tt}")
                        for k in range(KT):
                            nc.tensor.matmul(
                                ps[:],
                                xts[k][:, t4 * 128 : (t4 + 1) * 128],
                                wv[k][:],
                                start=(k == 0),
                                stop=(k == KT - 1),
                            )
                        st = p1_stage.tile([128, 512], mm_dt, tag="stv", name=f"vst{tt}")
                        nc.scalar.copy(st[:], ps[:])
                        nc.sync.dma_start(v_d[tt][:], st[:])

            # ---- Phase 2: attention, one head pair at a time ----
            with (
                tc.tile_pool(name="trig", bufs=1) as trig_pool,
                tc.tile_pool(name="eo", bufs=3) as eo_pool,
                tc.tile_pool(name="pch", bufs=3) as pch_pool,
                tc.tile_pool(name="rot", bufs=1) as rot_pool,
                tc.tile_pool(name="vt", bufs=16) as vt_pool,
                tc.tile_pool(name="tmp", bufs=4) as tmp_pool,
                tc.tile_pool(name="exp", bufs=4) as exp_pool,
                tc.tile_pool(name="sums", bufs=2) as sums_pool,
                tc.tile_pool(name="nrm", bufs=4) as nrm_pool,
                tc.tile_pool(name="sps", bufs=3, space="PSUM") as s_psum,
                tc.tile_pool(name="ops", bufs=2, space="PSUM") as o_psum,
                tc.tile_pool(name="rps", bufs=2, space="PSUM") as r_psum,
                tc.tile_pool(name="bps", bufs=1, space="PSUM") as b_psum,
            ):
                cos_sb = trig_pool.tile([128, T], _F32, tag="cos", name="cos_sb")
                sin_sb = trig_pool.tile([128, T], _F32, tag="sin", name="sin_sb")
                nc.gpsimd.dma_start(cos_sb[:], cos_d[:])
                nc.gpsimd.dma_start(sin_sb[:], sin_d[:])
                mask_sb = []
                for p in range(4):
                    msk = trig_pool.tile(
                        [128, 512], _F32, tag=f"mask{p}", name=f"mask{p}_sb"
                    )
                    nc.gpsimd.dma_start(msk[:], masks_d[p])
                    mask_sb.append(msk)

                # deferred normalization: the recip->broadcast->scale chain of
                # one (head, chunk) is emitted after the next chunk's first
                # score block so the PE never head-of-line stalls on it
                pending = [None]

                def flush_pending():
                    if pending[0] is not None:
                        pending[0]()
                        pending[0] = None

                for pr in range(2):  # head pair
                    rq = [
                        rot_pool.tile(
                            [128, T], mm_dt, tag=f"rq{s}", bufs=2,
                            name=f"rq{pr}_{s}",
                        )
                        for s in range(2)
                    ]
                    rk = [
                        rot_pool.tile(
                            [128, T], mm_dt, tag=f"rk{s}", bufs=2,
                            name=f"rk{pr}_{s}",
                        )
                        for s in range(2)
                    ]
                    # RoPE in pair layout (full-width DVE), then scatter the
                    # per-head halves into rq/rk via SBUF->SBUF DMA
                    for src_q, dsts in ((True, rq), (False, rk)):
                        mE = (0 if src_q else 4) + 2 * pr
                        for ch in range(TCH):
                            cs = slice(ch * 512, (ch + 1) * 512)
                            E = eo_pool.tile(
                                [128, 512], _F32, tag="E", bufs=3,
                                name=f"E{pr}_{src_q}_{ch}",
                            )
                            O = eo_pool.tile(
                                [128, 512], _F32, tag="O", bufs=3,
                                name=f"O{pr}_{src_q}_{ch}",
                            )
                            nc.gpsimd.dma_start(E[:], qk_d[mE][:, cs])
                            nc.gpsimd.dma_start(O[:], qk_d[mE + 1][:, cs])
                            P1 = pch_pool.tile(
                                [128, 512], mm_dt, tag="P1", bufs=3,
                                name=f"P1_{pr}_{src_q}_{ch}",
                            )
                            P2 = pch_pool.tile(
                                [128, 512], mm_dt, tag="P2", bufs=3,
                                name=f"P2_{pr}_{src_q}_{ch}",
                            )
                            tm = tmp_pool.tile(
                                [128, 512], _F32, tag="tmp", bufs=4, name="rt1"
                            )
                            nc.vector.tensor_mul(P1[:], E[:], cos_sb[:, cs])
                            nc.vector.tensor_mul(tm[:], O[:], sin_sb[:, cs])
                            nc.vector.tensor_sub(P1[:], P1[:], tm[:])
                            tm2 = tmp_pool.tile(
                                [128, 512], _F32, tag="tmp", bufs=4, name="rt2"
                            )
                            nc.vector.tensor_mul(P2[:], E[:], sin_sb[:, cs])
                            nc.vector.tensor_mul(tm2[:], O[:], cos_sb[:, cs])
                            nc.vector.tensor_add(P2[:], P2[:], tm2[:])
                            for s in range(2):
                                hs = slice(64 * s, 64 * s + 64)
                                nc.gpsimd.dma_start(dsts[s][0:64, cs], P1[hs, :])
                                nc.gpsimd.dma_start(dsts[s][64:128, cs], P2[hs, :])

                    for sub in range(2):  # head within pair
                        h = 2 * pr + sub
                        vts = []
                        for t in range(TT):
                            vt = vt_pool.tile(
                                [128, 128], mm_dt, tag="vt", bufs=16,
                                name=f"vt{h}_{t}",
                            )
                            nc.gpsimd.dma_start(
                                vt[:], v_d[t][:, h * 128 : (h + 1) * 128]
                            )
                            vts.append(vt)

                        for ic in range(TCH):
                            njt = 4 * ic + 4
                            ops = o_psum.tile(
                                [128, 512], _F32, tag="ops", name=f"ops{h}_{ic}"
                            )
                            sacc = sums_pool.tile(
                                [128, 512], mm_dt, tag="sacc", bufs=2,
                                name=f"sacc{h}_{ic}",
                            )
                            for jj in range(njt):
                                sps = s_psum.tile(
                                    [128, 512], _F32, tag="sps",
                                    name=f"sps{h}_{ic}_{jj}",
                                )
                                nc.tensor.matmul(
                                    sps[:],
                                    rk[sub][:, jj * 128 : (jj + 1) * 128],
                                    rq[sub][:, ic * 512 : (ic + 1) * 512],
                                    start=True,
                                    stop=True,
                                )
                                ex = exp_pool.tile(
                                    [128, 512], mm_dt, tag="exp", bufs=4,
                                    name=f"ex{h}_{ic}_{jj}",
                                )
                                nc.scalar.activation(
                                    ex[:], sps[:],
                                    mybir.ActivationFunctionType.Exp,
                                    scale=scale,
                                )
                                p = jj - 4 * ic
                                if p >= 0:
                                    exm = exp_pool.tile(
                                        [128, 512], mm_dt, tag="expm", bufs=4,
                                        name=f"exm{h}_{ic}_{jj}",
                                    )
                                    nc.vector.tensor_mul(
                                        exm[:], ex[:], mask_sb[p][:]
                                    )
                                    use = exm
                                else:
                                    use = ex
                                if jj == 0:
                                    nc.vector.tensor_copy(sacc[:], use[:])
                                else:
                                    nc.vector.tensor_add(sacc[:], sacc[:], use[:])
                                nc.tensor.matmul(
                                    ops[:],
                                    vts[jj][:],
                                    use[:],
                                    start=(jj == 0),
                                    stop=(jj == njt - 1),
                                )
                                if jj == 1:
                                    flush_pending()
                            rps = r_psum.tile(
                                [1, 512], _F32, tag="rps", bufs=2,
                                name=f"rps{h}_{ic}",
                            )
                            nc.tensor.matmul(
                                rps[:], ones_sb[:, 0:1], sacc[:],
                                start=True, stop=True,
                            )

                            def _norm(h=h, ic=ic, ops=ops, rps=rps):
                                rc32 = nrm_pool.tile(
                                    [1, 512], _F32, tag="rc32", bufs=2,
                                    name=f"rc32_{h}_{ic}",
                                )
                                nc.vector.reciprocal(rc32[:], rps[:])
                                rc = nrm_pool.tile(
                                    [1, 512], mm_dt, tag="rc", bufs=2,
                                    name=f"rc{h}_{ic}",
                                )
                                nc.scalar.copy(rc[:], rc32[:])
                                bps = b_psum.tile(
                                    [128, 512], _F32, tag="bps",
                                    name=f"bps{h}_{ic}",
                                )
                                nc.tensor.matmul(
                                    bps[:], ones_sb[0:1, :], rc[:],
                                    start=True, stop=True,
                                )
                                bcs = nrm_pool.tile(
                                    [128, 512], _F32, tag="bcs", bufs=2,
                                    name=f"bcs{h}_{ic}",
                                )
                                nc.scalar.copy(bcs[:], bps[:])
                                nc.vector.tensor_mul(
                                    attn[h][:, ic * 512 : (ic + 1) * 512],
                                    ops[:], bcs[:],
                                )

                            pending[0] = _norm
                flush_pending()

            # ---- Phase 3: partial out-projection ----
            with (
                tc.tile_pool(name="wout", bufs=4) as wout_pool,
                tc.tile_pool(name="p3ps", bufs=4, space="PSUM") as p3_psum,
                tc.tile_pool(name="p3st", bufs=4) as p3_stage,
            ):
                wout_sb = []
                for r in range(4):
                    w = wout_pool.tile(
                        [128, C], mm_dt, tag="wout", bufs=4, name=f"wout{r}"
                    )
                    nc.gpsimd.dma_start(w[:], woutt[r * 128 : (r + 1) * 128, :])
                    wout_sb.append(w)
                for t in range(TT):
                    for jo in range(TCH):
                        ps = p3_psum.tile(
                            [128, 512], _F32, tag="ps", name=f"ops3_{t}_{jo}"
                        )
                        for r in range(4):
                            nc.tensor.matmul(
                                ps[:],
                                attn[r][:, t * 128 : (t + 1) * 128],
                                wout_sb[r][:, jo * 512 : (jo + 1) * 512],
                                start=(r == 0),
                                stop=(r == 3),
                            )
                        st = p3_stage.tile(
                            [128, 512], _F32, tag="st", name=f"ost{t}_{jo}"
                        )
                        nc.scalar.copy(st[:], ps[:])
                        nc.sync.dma_start(
                            part[t * 128 : (t + 1) * 128,
                                 jo * 512 : (jo + 1) * 512],
                            st[:],
                        )

    _split_multiwait(nc)
    return nc


def _host_inputs(x, W_qkv, b_qkv, W_out):
    """Per-core input maps (host-side shard + transpose + tables)."""
    even = np.arange(0, D, 2)
    odd = np.arange(1, D, 2)

    inv_freq = 1.0 / (10000.0 ** (np.arange(0, D, 2, dtype=np.float64) / D))
    tpos = np.arange(T, dtype=np.float64)
    freqs = tpos[None, :] * inv_freq[:, None]  # [64, T]
    cos64 = np.cos(freqs)
    sin64 = np.sin(freqs)
    # duplicated per head pair: rows 0:64 head A, 64:128 head B
    cos = np.concatenate([cos64, cos64], axis=0).astype(np.float32)
    sin = np.concatenate([sin64, sin64], axis=0).astype(np.float32)

    masks = np.zeros((4, 128, 512), dtype=np.float32)
    jjj = np.arange(128)[:, None]
    iii = np.arange(512)[None, :]
    for p in range(4):
        masks[p] = (jjj + 128 * p <= iii).astype(np.float32)

    ones = np.ones((128, 128), dtype=np.float32)

    in_maps = []
    for c in range(NC):
        b, g = divmod(c, 4)
        # m-tiles: qE0 qO0 qE1 qO1 (pair-local heads (0,1),(2,3))
        def pair_rows(base):
            rows = []
            for pr in range(2):
                hA = base + 512 * g + 128 * (2 * pr)
                hB = base + 512 * g + 128 * (2 * pr + 1)
                rows.append(np.concatenate([hA + even, hB + even]))  # E tile
                rows.append(np.concatenate([hA + odd, hB + odd]))  # O tile
            return rows

        qk_rows = np.concatenate(pair_rows(0) + pair_rows(C))  # [2048]
        vrows = 2 * C + 512 * g + np.arange(512)
        rows = np.concatenate([qk_rows, vrows])
        wqkvt = np.ascontiguousarray(W_qkv[rows].T)  # [C, 1536]
        xt = np.ascontiguousarray(x[b].T)  # [C, T]
        woutt = np.ascontiguousarray(W_out[:, 512 * g : 512 * (g + 1)].T)
        bias_qk = np.ascontiguousarray(b_qkv[qk_rows].reshape(8, 128).T)
        in_maps.append(
            {
                "xt": xt,
                "wqkvt": wqkvt,
                "woutt": woutt,
                "cos": cos,
                "sin": sin,
                "masks": masks,
                "ones": ones,
                "bias_qk": bias_qk,
            }
        )
    return in_maps


def kernel(x, W_qkv, b_qkv, W_out, b_out, mm_dt="float32r", trace=False):
    x = np.asarray(x, dtype=np.float32)
    W_qkv = np.asarray(W_qkv, dtype=np.float32)
    b_qkv = np.asarray(b_qkv, dtype=np.float32)
    W_out = np.asarray(W_out, dtype=np.float32)
    b_out = np.asarray(b_out, dtype=np.float32)

    mm = mybir.dt.float32r if mm_dt == "float32r" else mybir.dt.float32
    with_qk_bias = bool(np.any(b_qkv[: 2 * C]))
    nc = _build_program(mm, with_qk_bias)
    in_maps = _host_inputs(x, W_qkv, b_qkv, W_out)

    kwargs = {}
    if trace:
        _ensure_ntff_hook()
        kwargs = dict(trace=True, trace_cores=[0])
    res = run_bass_kernel_spmd(nc, in_maps, core_ids=list(range(NC)), **kwargs)

    # host "all-reduce": sum the 4 partials per batch, add biases (the v-bias
    # passes through softmax exactly: attn rows sum to 1)
    corr = b_out + W_out @ b_qkv[2 * C :]
    out = np.empty((B, T, C), dtype=np.float32)
    for b in range(B):
        acc = res.results[4 * b]["part"].astype(np.float32)
        for c in range(4 * b + 1, 4 * b + 4):
            acc = acc + res.results[c]["part"]
        out[b] = acc + corr[None, :]

    if trace:
        return out, res.exec_time_ns
    return out


# revision 11
# speedup vs baseline: 1.2584x; 1.0191x over previous
"""Trainium2 Bass kernel: causal multi-head attention with RoPE + out-proj.

Problem shapes (hardcoded): x [2, 2048, 2048], W_qkv [6144, 2048],
b_qkv [6144], W_out [2048, 2048], b_out [2048]. H=16 heads, D=128.

Sharding over 8 NeuronCores: core c handles batch b = c//4 and head group
g = c%4 (4 heads). Each core computes its heads' attention output and a
partial out-projection over its 512 columns of the contraction; the host
sums the 4 partials per batch (the "all-reduce") and adds biases.

Device-side layout choices (matmuls are [K,M]@[K,N], N=512 free dim):
- host pre-transposes x and the weight shards so contractions land on
  partitions with contiguous DMA reads
- q/k are produced transposed ([dim, token]) in head PAIRS: one 128-row
  tile holds the even dims of both heads, another the odd dims, so RoPE
  is full-width 128-partition DVE math with same-start-partition
  operands (walrus's TensorTensor verifier rejects mixed partition
  offsets); the scores contraction then splits into two K=64 matmuls
  (first-half dims + second-half dims) accumulating in PSUM
- v is produced in natural layout ([token, dim]) so it can be the
  stationary operand of the attention@V matmul
- scores are computed transposed (keys on partitions, queries free);
  softmax skips the max-subtraction (scores here are ~N(0,1), exp can't
  overflow in fp32) so exp needs no per-row bias, the causal mask is a
  multiplicative 0/1 mask after exp, and the denominator comes from a
  ones-vector matmul over the accumulated exp tiles
"""

import math

import numpy as np

import concourse.bass as bass
import concourse.mybir as mybir
from concourse.bass_utils import run_bass_kernel_spmd
from concourse.tile import TileContext
from concourse.vector_clock import ScopedClock

B, T, C = 2, 2048, 2048
H, D = 16, 128
G = 4  # heads per core
NC = 8  # cores
KT = C // 128  # 16 contraction tiles
TT = T // 128  # 16 token tiles
TCH = T // 512  # 4 token chunks

_F32 = mybir.dt.float32


def _install_drain_patch():
    """Walrus in this env rejects >1 sync wait on one CTRL (Drain) inst.

    Tile's tail drain attaches one wait per outstanding logical proc to a
    single Drain; split them across single-wait NOPs on SP instead.
    """

    def _drain_and_barrier(self, tick_clock, wait_clock):
        probe = self.nc.sync.nop(nofuse=True)
        wait_clock.add_sem_waits(
            probe.ins, ScopedClock({None: tick_clock.global_clock})
        )
        si = probe.ins.sync_info
        if si is not None and len(si.on_wait) > 1:
            waits = list(si.on_wait)
            probe.ins.sync_info = mybir.SyncInfo(
                on_wait=waits[:1], on_update=list(si.on_update)
            )
            for w in waits[1:]:
                extra = self.nc.sync.nop(nofuse=True)
                extra.ins.sync_info = mybir.SyncInfo(on_wait=[w], on_update=[])
        self.nc.sync.drain()
        self.nc.all_engine_barrier()
        popped = self.nc._tile_sem_poison_stack.pop()
        assert popped is self._sem_poison
        self.nc.clear_and_free_semaphores(list(self.sems.allocated().values()))
        self.nc.all_engine_barrier()

    TileContext._drain_and_barrier = _drain_and_barrier


_install_drain_patch()


def _split_multiwait(nc):
    """Walrus here allows only one sync wait per instruction: move extras
    onto single-wait NOPs inserted just before, on the same engine."""
    for fn in nc.m.functions:
        for bb in fn.blocks:
            insts = bb.instructions
            new = []
            changed = False
            for inst in insts:
                si = inst.sync_info
                if si is not None and len(si.on_wait) > 1:
                    waits = list(si.on_wait)
                    for w in waits[:-1]:
                        nop = mybir.InstNoOp(
                            name=nc.get_next_instruction_name(), ins=[], outs=[]
                        )
                        nop.engine = inst.engine
                        nop.sync_info = mybir.SyncInfo(on_wait=[w], on_update=[])
                        new.append(nop)
                    inst.sync_info = mybir.SyncInfo(
                        on_wait=[waits[-1]], on_update=list(si.on_update)
                    )
                    changed = True
                new.append(inst)
            if changed:
                insts[:] = new


def _ensure_ntff_hook():
    """This image's antenv lacks axon_hooks; graft a minimal one so
    trace=True can reach the NTFF profiler instead of crashing."""
    import sys
    import types

    try:
        import antenv.axon_hooks  # noqa: F401

        return
    except ImportError:
        pass
    import antenv
    from trn_agent_boot.trn_boot import _ntff_profile_via_ctypes

    mod = types.ModuleType("antenv.axon_hooks")
    _h = [None]
    mod.set_axon_ntff_profile_hook = lambda h: _h.__setitem__(0, h)
    mod.get_axon_ntff_profile_hook = lambda: _h[0]
    sys.modules["antenv.axon_hooks"] = mod
    antenv.axon_hooks = mod
    try:
        mod.set_axon_ntff_profile_hook(
            _ntff_profile_via_ctypes("/opt/axon/libaxon_pjrt.so")
        )
    except Exception:
        pass


def _build_program(mm_dt, with_qk_bias: bool):
    nc = bass.Bass()

    xt = nc.dram_tensor("xt", [C, T], mm_dt, kind="ExternalInput")
    wqkvt = nc.dram_tensor("wqkvt", [C, 3 * 512], mm_dt, kind="ExternalInput")
    woutt = nc.dram_tensor("woutt", [512, C], mm_dt, kind="ExternalInput")
    cos_d = nc.dram_tensor("cos", [128, T], _F32, kind="ExternalInput")
    sin_d = nc.dram_tensor("sin", [128, T], _F32, kind="ExternalInput")
    masks_d = nc.dram_tensor("masks", [4, 128, 512], _F32, kind="ExternalInput")
    ones_d = nc.dram_tensor("ones", [128, 128], mm_dt, kind="ExternalInput")
    bias_d = nc.dram_tensor("bias_qk", [128, 8], _F32, kind="ExternalInput")
    part = nc.dram_tensor("part", [T, C], _F32, kind="ExternalOutput")

    scale = 1.0 / math.sqrt(D)

    with TileContext(nc) as tc:
        with (
            tc.tile_pool(name="consts", bufs=1) as cpool,
            tc.tile_pool(name="attn", bufs=1) as apool,
            tc.tile_pool(name="dram", bufs=1, space="DRAM") as dpool,
        ):
            ones_sb = cpool.tile([128, 128], mm_dt, tag="ones", name="ones_sb")
            nc.sync.dma_start(ones_sb[:], ones_d[:])
            if with_qk_bias:
                bias_sb = cpool.tile([128, 8], _F32, tag="bias", name="bias_sb")
                nc.sync.dma_start(bias_sb[:], bias_d[:])

            # m-tile order in wqkvt / qk_d: qE0 qO0 qE1 qO1 kE0 kO0 kE1 kO1
            # (pair p: even dims of heads 2p,2p+1 / odd dims), then v 512.
            qk_d = [
                [
                    dpool.tile(
                        [128, 512], _F32, tag=f"qk{m}_{ch}", name=f"qk{m}_{ch}_d"
                    )
                    for ch in range(TCH)
                ]
                for m in range(8)
            ]
            v_d = [
                dpool.tile([128, 512], mm_dt, tag=f"v{t}", name=f"v{t}_d")
                for t in range(TT)
            ]
            attn = [
                apool.tile([128, T], mm_dt, tag=f"attn{r}", name=f"attn{r}_sb")
                for r in range(4)
            ]

            # ---- Phase 1: qkv projection ----
            with (
                tc.tile_pool(name="wqk", bufs=16) as wqk_pool,
                tc.tile_pool(name="wv", bufs=16) as wv_pool,
                tc.tile_pool(name="xch", bufs=16) as x_pool,
                tc.tile_pool(name="p1ps", bufs=4, space="PSUM") as p1_psum,
                tc.tile_pool(name="p1st", bufs=3) as p1_stage,
            ):
                wqk = []
                wv = []
                for k in range(KT):
                    wk = wqk_pool.tile(
                        [128, 1024], mm_dt, tag="wqk", bufs=16, name=f"wqk{k}"
                    )
                    nc.sync.dma_start(wk[:], wqkvt[k * 128 : (k + 1) * 128, 0:1024])
                    wqk.append(wk)
                    wvk = wv_pool.tile(
                        [128, 512], mm_dt, tag="wv", bufs=16, name=f"wv{k}"
                    )
                    nc.sync.dma_start(
                        wvk[:], wqkvt[k * 128 : (k + 1) * 128, 1024:1536]
                    )
                    wv.append(wvk)

                for n in range(TCH):
                    xts = []
                    for k in range(KT):
                        xk = x_pool.tile(
                            [128, 512], mm_dt, tag="x", bufs=16, name=f"x{n}_{k}"
                        )
                        nc.sync.dma_start(
                            xk[:],
                            xt[k * 128 : (k + 1) * 128, n * 512 : (n + 1) * 512],
                        )
                        xts.append(xk)
                    for m in range(8):
                        ps = p1_psum.tile(
                            [128, 512], _F32, tag="ps", name=f"qkps{n}_{m}"
                        )
                        for k in range(KT):
                            nc.tensor.matmul(
                                ps[:],
                                wqk[k][:, m * 128 : (m + 1) * 128],
                                xts[k][:],
                                start=(k == 0),
                                stop=(k == KT - 1),
                            )
                        st = p1_stage.tile(
                            [128, 512], _F32, tag="st", name=f"qkst{n}_{m}"
                        )
                        if with_qk_bias:
                            nc.vector.tensor_scalar_add(
                                st[:], ps[:], bias_sb[:, m : m + 1]
                            )
                        else:
                            nc.scalar.copy(st[:], ps[:])
                        nc.sync.dma_start(qk_d[m][n][:], st[:])
                    for t4 in range(4):
                        tt = n * 4 + t4
                        ps = p1_psum.tile([128, 512], _F32, tag="ps", name=f"vps{tt}")
                        for k in range(KT):
                            nc.tensor.matmul(
                                ps[:],
                                xts[k][:, t4 * 128 : (t4 + 1) * 128],
                                wv[k][:],
                                start=(k == 0),
                                stop=(k == KT - 1),
                            )
                        st = p1_stage.tile([128, 512], mm_dt, tag="stv", name=f"vst{tt}")
                        nc.scalar.copy(st[:], ps[:])
                        nc.sync.dma_start(v_d[tt][:], st[:])

            # ---- Phase 2: attention, one head pair at a time ----
            with (
                tc.tile_pool(name="trig", bufs=1) as trig_pool,
                tc.tile_pool(name="eo", bufs=3) as eo_pool,
                tc.tile_pool(name="pch", bufs=3) as pch_pool,
                tc.tile_pool(name="rot", bufs=1) as rot_pool,
                tc.tile_pool(name="vt", bufs=16) as vt_pool,
                tc.tile_pool(name="tmp", bufs=4) as tmp_pool,
                tc.tile_pool(name="exp", bufs=4) as exp_pool,
                tc.tile_pool(name="sums", bufs=2) as sums_pool,
                tc.tile_pool(name="nrm", bufs=4) as nrm_pool,
                tc.tile_pool(name="sps", bufs=3, space="PSUM") as s_psum,
                tc.tile_pool(name="bps", bufs=1, space="PSUM") as b_psum,
                tc.tile_pool(name="ops", bufs=3, space="PSUM") as o_psum,
                tc.tile_pool(name="rps", bufs=1, space="PSUM") as r_psum,
            ):
                cos_sb = trig_pool.tile([128, T], _F32, tag="cos", name="cos_sb")
                sin_sb = trig_pool.tile([128, T], _F32, tag="sin", name="sin_sb")
                nc.gpsimd.dma_start(cos_sb[:], cos_d[:])
                nc.gpsimd.dma_start(sin_sb[:], sin_d[:])
                mask_sb = []
                for p in range(4):
                    msk = trig_pool.tile(
                        [128, 512], _F32, tag=f"mask{p}", name=f"mask{p}_sb"
                    )
                    nc.gpsimd.dma_start(msk[:], masks_d[p])
                    mask_sb.append(msk)

                # deferred normalization: the recip->broadcast->scale chain of
                # one (head, chunk) is emitted after the next chunk's first
                # score block so the PE never head-of-line stalls on it
                pendings = []

                def flush_pending(depth=2):
                    while len(pendings) >= depth:
                        pendings.pop(0)()

                def flush_all():
                    while pendings:
                        pendings.pop(0)()

                for pr in range(2):  # head pair
                    rq = [
                        rot_pool.tile(
                            [128, T], mm_dt, tag=f"rq{s}", bufs=2,
                            name=f"rq{pr}_{s}",
                        )
                        for s in range(2)
                    ]
                    rk = [
                        rot_pool.tile(
                            [128, T], mm_dt, tag=f"rk{s}", bufs=2,
                            name=f"rk{pr}_{s}",
                        )
                        for s in range(2)
                    ]
                    # RoPE in pair layout (full-width DVE), then scatter the
                    # per-head halves into rq/rk via SBUF->SBUF DMA
                    for src_q, dsts in ((True, rq), (False, rk)):
                        mE = (0 if src_q else 4) + 2 * pr
                        for ch in range(TCH):
                            cs = slice(ch * 512, (ch + 1) * 512)
                            E = eo_pool.tile(
                                [128, 512], _F32, tag="E", bufs=3,
                                name=f"E{pr}_{src_q}_{ch}",
                            )
                            O = eo_pool.tile(
                                [128, 512], _F32, tag="O", bufs=3,
                                name=f"O{pr}_{src_q}_{ch}",
                            )
                            nc.gpsimd.dma_start(E[:], qk_d[mE][ch][:])
                            nc.gpsimd.dma_start(O[:], qk_d[mE + 1][ch][:])
                            P1 = pch_pool.tile(
                                [128, 512], mm_dt, tag="P1", bufs=3,
                                name=f"P1_{pr}_{src_q}_{ch}",
                            )
                            P2 = pch_pool.tile(
                                [128, 512], mm_dt, tag="P2", bufs=3,
                                name=f"P2_{pr}_{src_q}_{ch}",
                            )
                            tm = tmp_pool.tile(
                                [128, 512], _F32, tag="tmp", bufs=4, name="rt1"
                            )
                            nc.vector.tensor_mul(P1[:], E[:], cos_sb[:, cs])
                            nc.vector.tensor_mul(tm[:], O[:], sin_sb[:, cs])
                            nc.vector.tensor_sub(P1[:], P1[:], tm[:])
                            tm2 = tmp_pool.tile(
                                [128, 512], _F32, tag="tmp", bufs=4, name="rt2"
                            )
                            nc.vector.tensor_mul(P2[:], E[:], sin_sb[:, cs])
                            nc.vector.tensor_mul(tm2[:], O[:], cos_sb[:, cs])
                            nc.vector.tensor_add(P2[:], P2[:], tm2[:])
                            for s in range(2):
                                hs = slice(64 * s, 64 * s + 64)
                                nc.gpsimd.dma_start(dsts[s][0:64, cs], P1[hs, :])
                                nc.gpsimd.dma_start(dsts[s][64:128, cs], P2[hs, :])

                    for sub in range(2):  # head within pair
                        h = 2 * pr + sub
                        vts = []
                        for t in range(TT):
                            vt = vt_pool.tile(
                                [128, 128], mm_dt, tag="vt", bufs=16,
                                name=f"vt{h}_{t}",
                            )
                            nc.gpsimd.dma_start(
                                vt[:], v_d[t][:, h * 128 : (h + 1) * 128]
                            )
                            vts.append(vt)

                        for ic in range(TCH):
                            njt = 4 * ic + 4
                            ops = o_psum.tile(
                                [128, 512], _F32, tag="ops", name=f"ops{h}_{ic}"
                            )
                            sacc = sums_pool.tile(
                                [128, 512], mm_dt, tag="sacc", bufs=2,
                                name=f"sacc{h}_{ic}",
                            )
                            for jj in range(njt):
                                sps = s_psum.tile(
                                    [128, 512], _F32, tag="sps",
                                    name=f"sps{h}_{ic}_{jj}",
                                )
                                nc.tensor.matmul(
                                    sps[:],
                                    rk[sub][:, jj * 128 : (jj + 1) * 128],
                                    rq[sub][:, ic * 512 : (ic + 1) * 512],
                                    start=True,
                                    stop=True,
                                )
                                ex = exp_pool.tile(
                                    [128, 512], mm_dt, tag="exp", bufs=4,
                                    name=f"ex{h}_{ic}_{jj}",
                                )
                                nc.scalar.activation(
                                    ex[:], sps[:],
                                    mybir.ActivationFunctionType.Exp,
                                    scale=scale,
                                )
                                p = jj - 4 * ic
                                if p >= 0:
                                    exm = exp_pool.tile(
                                        [128, 512], mm_dt, tag="expm", bufs=4,
                                        name=f"exm{h}_{ic}_{jj}",
                                    )
                                    nc.vector.tensor_mul(
                                        exm[:], ex[:], mask_sb[p][:]
                                    )
                                    use = exm
                                else:
                                    use = ex
                                if jj == 0:
                                    nc.vector.tensor_copy(sacc[:], use[:])
                                else:
                                    nc.vector.tensor_add(sacc[:], sacc[:], use[:])
                                nc.tensor.matmul(
                                    ops[:],
                                    vts[jj][:],
                                    use[:],
                                    start=(jj == 0),
                                    stop=(jj == njt - 1),
                                )
                                if jj == 1:
                                    flush_pending(depth=2)
                            rps = r_psum.tile(
                                [1, 512], _F32, tag="rps", bufs=1,
                                name=f"rps{h}_{ic}",
                            )
                            nc.tensor.matmul(
                                rps[:], ones_sb[:, 0:1], sacc[:],
                                start=True, stop=True,
                            )
                            rc = nrm_pool.tile(
                                [1, 512], mm_dt, tag="rc", bufs=3,
                                name=f"rc{h}_{ic}",
                            )
                            with nc.allow_low_precision(
                                reason="softmax denominators fit fp32r"
                            ):
                                nc.vector.reciprocal(rc[:], rps[:])

                            def _norm(h=h, ic=ic, ops=ops, rc=rc):
                                bps = b_psum.tile(
                                    [128, 512], _F32, tag="bps",
                                    name=f"bps{h}_{ic}",
                                )
                                nc.tensor.matmul(
                                    bps[:], ones_sb[0:1, :], rc[:],
                                    start=True, stop=True,
                                )
                                bcs = nrm_pool.tile(
                                    [128, 512], _F32, tag="bcs", bufs=2,
                                    name=f"bcs{h}_{ic}",
                                )
                                nc.scalar.copy(bcs[:], bps[:])
                                nc.vector.tensor_mul(
                                    attn[h][:, ic * 512 : (ic + 1) * 512],
                                    ops[:], bcs[:],
                                )

                            pendings.append(_norm)
                flush_all()

            # ---- Phase 3: partial out-projection ----
            with (
                tc.tile_pool(name="wout", bufs=4) as wout_pool,
                tc.tile_pool(name="p3ps", bufs=4, space="PSUM") as p3_psum,
                tc.tile_pool(name="p3st", bufs=4) as p3_stage,
            ):
                wout_sb = []
                for r in range(4):
                    w = wout_pool.tile(
                        [128, C], mm_dt, tag="wout", bufs=4, name=f"wout{r}"
                    )
                    nc.gpsimd.dma_start(w[:], woutt[r * 128 : (r + 1) * 128, :])
                    wout_sb.append(w)
                for t in range(TT):
                    for jo in range(TCH):
                        ps = p3_psum.tile(
                            [128, 512], _F32, tag="ps", name=f"ops3_{t}_{jo}"
                        )
                        for r in range(4):
                            nc.tensor.matmul(
                                ps[:],
                                attn[r][:, t * 128 : (t + 1) * 128],
                                wout_sb[r][:, jo * 512 : (jo + 1) * 512],
                                start=(r == 0),
                                stop=(r == 3),
                            )
                        st = p3_stage.tile(
                            [128, 512], _F32, tag="st", name=f"ost{t}_{jo}"
                        )
                        nc.scalar.copy(st[:], ps[:])
                        nc.sync.dma_start(
                            part[t * 128 : (t + 1) * 128,
                                 jo * 512 : (jo + 1) * 512],
                            st[:],
                        )

    _split_multiwait(nc)
    return nc


def _host_inputs(x, W_qkv, b_qkv, W_out):
    """Per-core input maps (host-side shard + transpose + tables)."""
    even = np.arange(0, D, 2)
    odd = np.arange(1, D, 2)

    inv_freq = 1.0 / (10000.0 ** (np.arange(0, D, 2, dtype=np.float64) / D))
    tpos = np.arange(T, dtype=np.float64)
    freqs = tpos[None, :] * inv_freq[:, None]  # [64, T]
    cos64 = np.cos(freqs)
    sin64 = np.sin(freqs)
    # duplicated per head pair: rows 0:64 head A, 64:128 head B
    cos = np.concatenate([cos64, cos64], axis=0).astype(np.float32)
    sin = np.concatenate([sin64, sin64], axis=0).astype(np.float32)

    masks = np.zeros((4, 128, 512), dtype=np.float32)
    jjj = np.arange(128)[:, None]
    iii = np.arange(512)[None, :]
    for p in range(4):
        masks[p] = (jjj + 128 * p <= iii).astype(np.float32)

    ones = np.ones((128, 128), dtype=np.float32)

    in_maps = []
    for c in range(NC):
        b, g = divmod(c, 4)
        # m-tiles: qE0 qO0 qE1 qO1 (pair-local heads (0,1),(2,3))
        def pair_rows(base):
            rows = []
            for pr in range(2):
                hA = base + 512 * g + 128 * (2 * pr)
                hB = base + 512 * g + 128 * (2 * pr + 1)
                rows.append(np.concatenate([hA + even, hB + even]))  # E tile
                rows.append(np.concatenate([hA + odd, hB + odd]))  # O tile
            return rows

        qk_rows = np.concatenate(pair_rows(0) + pair_rows(C))  # [2048]
        vrows = 2 * C + 512 * g + np.arange(512)
        rows = np.concatenate([qk_rows, vrows])
        wqkvt = np.ascontiguousarray(W_qkv[rows].T)  # [C, 1536]
        xt = np.ascontiguousarray(x[b].T)  # [C, T]
        woutt = np.ascontiguousarray(W_out[:, 512 * g : 512 * (g + 1)].T)
        bias_qk = np.ascontiguousarray(b_qkv[qk_rows].reshape(8, 128).T)
        in_maps.append(
            {
                "xt": xt,
                "wqkvt": wqkvt,
                "woutt": woutt,
                "cos": cos,
                "sin": sin,
                "masks": masks,
                "ones": ones,
                "bias_qk": bias_qk,
            }
        )
    return in_maps


def kernel(x, W_qkv, b_qkv, W_out, b_out, mm_dt="float32r", trace=False):
    x = np.asarray(x, dtype=np.float32)
    W_qkv = np.asarray(W_qkv, dtype=np.float32)
    b_qkv = np.asarray(b_qkv, dtype=np.float32)
    W_out = np.asarray(W_out, dtype=np.float32)
    b_out = np.asarray(b_out, dtype=np.float32)

    mm = mybir.dt.float32r if mm_dt == "float32r" else mybir.dt.float32
    with_qk_bias = bool(np.any(b_qkv[: 2 * C]))
    nc = _build_program(mm, with_qk_bias)
    in_maps = _host_inputs(x, W_qkv, b_qkv, W_out)

    kwargs = {}
    if trace:
        _ensure_ntff_hook()
        kwargs = dict(trace=True, trace_cores=[0])
    res = run_bass_kernel_spmd(nc, in_maps, core_ids=list(range(NC)), **kwargs)

    # host "all-reduce": sum the 4 partials per batch, add biases (the v-bias
    # passes through softmax exactly: attn rows sum to 1)
    corr = b_out + W_out @ b_qkv[2 * C :]
    out = np.empty((B, T, C), dtype=np.float32)
    for b in range(B):
        acc = res.results[4 * b]["part"].astype(np.float32)
        for c in range(4 * b + 1, 4 * b + 4):
            acc = acc + res.results[c]["part"]
        out[b] = acc + corr[None, :]

    if trace:
        return out, res.exec_time_ns
    return out
